# revision 26
# baseline (speedup 1.0000x reference)
"""Trainium2 Bass kernel for MHSA with Transformer-XL relative position bias.

Problem: B=16, T=1024, DM=256, H=4, HS=64 fp32.
Sharding: pure data-parallel over batch across 8 cores (2 batches/core).

v2 design (fp8 DoubleRow everywhere upstream):
  - LN in f32 -> xn bf16 -> PE transpose bf16 -> xnT8/posT8 fp8 [128, 2dc, M]
    (d packed as 2 k-subtiles for DoubleRow).
  - Projections: one fp8-DR matmul per psum; W columns host-permuted so the
    q/k head-fold layout [32h+k partitions, j subtile] falls out of the psum.
    Weights scaled x16 on host (fp8 subnormal avoidance), 1/16 folded into
    the psum->fp8 copies.
  - Scores per (b,h): X = qv.P^T and C = qu.K^T via fp8-DR [K=32x2] matmuls;
    X -> bf16 -> DRAM scratch [1024,1025] (col0 = 0); rel_shift = strided
    re-read (flat-buffer shear). R added into C-psum via bf16 identity
    matmul; ACT exp from psum -> E fp8 + exact f32 row-sum accum.
  - E^T via xbar DMA transpose of E viewed as u16 pairs -> at [128,4c,1024m]
    (logical row r = c*128+p => n = 256c+2p+j). V stored pre-packed to the
    matching layout; AV = fp8-DR over (c, j) -> avps [64, 1024] psum.
  - Softmax normalization deferred: row-sums -> recip -> PE-transpose ->
    fold -> partition_broadcast -> recB [128, m]; avT = avps * recB (bf16).
  - Out-proj bf16 from avT + residual + bo.
"""
import sys

sys.path.insert(0, "/opt/trn_rl_repo")

import numpy as np

import concourse.bass as bass
import concourse.bacc as bacc
import concourse.tile as tile
from concourse import mybir
from concourse.masks import make_identity
from concourse.bass_utils import run_bass_kernel_spmd

B, T, DM, H, HS = 16, 1024, 256, 4, 64
NCORES = 8
BL = B // NCORES          # local batches per core (2)
M = BL * T                # local rows (2048)
NMT = M // 128            # m-tiles (16)
P = 128
NBH = BL * H              # 8 (b,h) pairs per core
NSCR = 4                  # rotating DRAM scratch buffers
LN_EPS = 1e-3
WSCALE = 16.0             # host-side weight scale (fp8 subnormal avoidance)
F32 = mybir.dt.float32
BF16 = mybir.dt.bfloat16
FP8 = mybir.dt.float8e4
U16 = mybir.dt.uint16
DR = mybir.MatmulPerfMode.DoubleRow
EXP = mybir.ActivationFunctionType.Exp
IDENT = mybir.ActivationFunctionType.Identity
SQRT = mybir.ActivationFunctionType.Sqrt
ADD = mybir.AluOpType.add
MULT = mybir.AluOpType.mult
SUB = mybir.AluOpType.subtract
BYPASS = mybir.AluOpType.bypass


def build_bass():
    nc = bacc.Bacc("TRN2", target_bir_lowering=False, debug=False,
                   enable_asserts=False, num_devices=NCORES)

    x_in = nc.dram_tensor("x", [M, DM], F32, kind="ExternalInput").ap()
    pos_in = nc.dram_tensor("pos", [M, DM], F32, kind="ExternalInput").ap()
    wq_in = nc.dram_tensor("wq", [2, P, DM], F32, kind="ExternalInput").ap()
    wk_in = nc.dram_tensor("wk", [2, P, DM], F32, kind="ExternalInput").ap()
    wp_in = nc.dram_tensor("wp", [2, P, DM], F32, kind="ExternalInput").ap()
    wv_in = nc.dram_tensor("wv", [2, P, DM], F32, kind="ExternalInput").ap()
    wo_in = nc.dram_tensor("wo", [4, 64, DM], F32, kind="ExternalInput").ap()
    bvec_in = nc.dram_tensor("bvec", [6, P], F32, kind="ExternalInput").ap()
    bo_in = nc.dram_tensor("bo", [DM], F32, kind="ExternalInput").ap()
    out = nc.dram_tensor("out", [M, DM], F32, kind="ExternalOutput").ap()

    scr = [
        nc.dram_tensor(f"xscr{i}", [T, T + 1], BF16, kind="Internal").ap()
        for i in range(NSCR)
    ]

    with tile.TileContext(nc) as tc:
        with tc.tile_pool(name="persist", bufs=1) as pp:
            # --- persistent SBUF ---
            ident = pp.tile([P, P], F32)
            make_identity(nc, ident)
            id_bf = pp.tile([P, P], BF16)
            nc.gpsimd.tensor_copy(out=id_bf, in_=ident)

            def load_w8(ap_in, name):
                tmp = pp.tile([P, 2, DM], F32, tag=f"{name}t", name=f"{name}t")
                for dc in range(2):
                    nc.sync.dma_start(out=tmp[:, dc, :], in_=ap_in[dc])
                w8 = pp.tile([P, 2, DM], FP8, tag=name, name=name)
                nc.vector.tensor_copy(out=w8, in_=tmp)
                return w8

            w8q = load_w8(wq_in, "w8q")
            w8k = load_w8(wk_in, "w8k")
            w8p = load_w8(wp_in, "w8p")
            w8v = load_w8(wv_in, "w8v")

            wo_sb = []
            for h_ in range(4):
                tmp = pp.tile([64, DM], F32, tag=f"wot{h_}", name=f"wot{h_}")
                nc.sync.dma_start(out=tmp, in_=wo_in[h_])
                t = pp.tile([64, DM], BF16, tag=f"wo{h_}", name=f"wo{h_}")
                nc.scalar.copy(out=t, in_=tmp)
                wo_sb.append(t)

            def load_col(row, name):
                t = pp.tile([P, 1], F32, tag=name, name=name)
                nc.sync.dma_start(
                    out=t,
                    in_=bass.AP(tensor=bvec_in.tensor, offset=row * P,
                                ap=[[1, P], [1, 1]]),
                )
                return t

            bqu_c = [load_col(0, "bqu0"), load_col(1, "bqu1")]
            dqv_c = [load_col(2, "dqv0"), load_col(3, "dqv1")]
            bk_c = [load_col(4, "bk0"), load_col(5, "bk1")]

            bo_b = pp.tile([P, DM], F32, tag="bo_b", name="bo_b")
            nc.sync.dma_start(
                out=bo_b,
                in_=bass.AP(tensor=bo_in.tensor, offset=0, ap=[[0, P], [1, DM]]),
            )

            eps_t = pp.tile([P, 1], F32)
            nc.vector.memset(eps_t, LN_EPS)

            x_res = pp.tile([P, NMT, DM], F32)
            xnT8 = pp.tile([P, 2, M], FP8)
            posT8 = pp.tile([P, 2, M], FP8)
            qu8 = [pp.tile([P, M], FP8, tag=f"qu8{c}", name=f"qu8{c}")
                   for c in range(2)]
            qv8 = [pp.tile([P, M], FP8, tag=f"qv8{c}", name=f"qv8{c}")
                   for c in range(2)]
            k8 = [pp.tile([P, M], FP8, tag=f"k8{c}", name=f"k8{c}")
                  for c in range(2)]
            p8 = [pp.tile([P, M], FP8, tag=f"p8{c}", name=f"p8{c}")
                  for c in range(2)]
            # V packed for AV DoubleRow: v8p[p, b, c, j, s] = V[b, 256c+2p+j, s]
            v8p = pp.tile([P, BL, 4, 2, DM], FP8)
            avT = [pp.tile([64, M], BF16, tag=f"avT{c}", name=f"avT{c}")
                   for c in range(4)]

            xbf_ring = [pp.tile([P, 2, T + 1], BF16, tag=f"xbf{i}",
                                name=f"xbf{i}") for i in range(3)]
            for t in xbf_ring:
                nc.gpsimd.memset(t[:, :, 0:1], 0.0)

            # ---------------- phase 1+2: LN, transposes, projections --------
            with tc.tile_pool(name="ph1", bufs=3) as sb1, \
                 tc.tile_pool(name="ps1", bufs=2, space="PSUM") as ps1, \
                 tc.tile_pool(name="ps2", bufs=2, space="PSUM") as ps2, \
                 tc.tile_pool(name="psv", bufs=2, space="PSUM") as psv:

                def ph1_body(mt):
                    xs = x_res[:, mt, :]
                    nc.sync.dma_start(out=xs, in_=x_in[mt * P:(mt + 1) * P, :])
                    stats = sb1.tile([P, 6], F32, tag="stats")
                    nc.vector.bn_stats(out=stats, in_=xs)
                    mv = sb1.tile([P, 2], F32, tag="mv")
                    nc.vector.bn_aggr(out=mv, in_=stats)
                    rstd = sb1.tile([P, 1], F32, tag="rstd")
                    nc.scalar.activation(out=rstd, in_=mv[:, 1:2], func=SQRT,
                                         bias=eps_t, scale=1.0)
                    nc.vector.reciprocal(out=rstd, in_=rstd)
                    xnb = sb1.tile([P, DM], BF16, tag="xnb")
                    nc.vector.tensor_scalar(out=xnb, in0=xs,
                                            scalar1=mv[:, 0:1], scalar2=rstd,
                                            op0=SUB, op1=MULT)
                    pt = sb1.tile([P, DM], F32, tag="pt")
                    nc.sync.dma_start(out=pt, in_=pos_in[mt * P:(mt + 1) * P, :])
                    pb = sb1.tile([P, DM], BF16, tag="pb")
                    nc.gpsimd.tensor_copy(out=pb, in_=pt)
                    msl = slice(mt * P, (mt + 1) * P)
                    for c in range(2):
                        tp = ps1.tile([P, P], BF16, tag="tp")
                        nc.tensor.transpose(tp, xnb[:, c * P:(c + 1) * P], id_bf)
                        nc.scalar.copy(out=xnT8[:, c, msl], in_=tp)
                        tq = ps1.tile([P, P], BF16, tag="tp", name="tq")
                        nc.tensor.transpose(tq, pb[:, c * P:(c + 1) * P], id_bf)
                        nc.vector.tensor_copy(out=posT8[:, c, msl], in_=tq)

                def ph2_chunk(mc):
                    msl = slice(mc * 512, (mc + 1) * 512)
                    ivw = 1.0 / WSCALE

                    def proj(w8, rhsT, sc_):
                        pa = ps2.tile([P, 512], F32, tag="prA", name="prA")
                        nc.tensor.matmul(pa, lhsT=w8[:, :, sc_ * P:(sc_ + 1) * P],
                                         rhs=rhsT[:, :, msl],
                                         start=True, stop=True, perf_mode=DR)
                        return pa

                    for sc_ in range(2):
                        qp = proj(w8q, xnT8, sc_)
                        nc.scalar.activation(out=qu8[sc_][:, msl], in_=qp,
                                             func=IDENT, bias=bqu_c[sc_],
                                             scale=ivw)
                        nc.vector.tensor_scalar_add(out=qv8[sc_][:, msl],
                                                    in0=qu8[sc_][:, msl],
                                                    scalar1=dqv_c[sc_])
                        kp = proj(w8k, xnT8, sc_)
                        nc.vector.tensor_scalar(out=k8[sc_][:, msl], in0=kp,
                                                scalar1=ivw, scalar2=bk_c[sc_],
                                                op0=MULT, op1=ADD)
                        pp_ = proj(w8p, posT8, sc_)
                        nc.scalar.mul(p8[sc_][:, msl], pp_, ivw)
                    for mt in range(mc * 4, mc * 4 + 4):
                        pv = psv.tile([P, DM], F32, tag="pv")
                        nc.tensor.matmul(
                            pv, lhsT=xnT8[:, :, mt * P:(mt + 1) * P],
                            rhs=w8v, start=True, stop=True, perf_mode=DR)
                        v8s = sb1.tile([P, DM], FP8, tag="v8s")
                        if mt % 2 == 0:
                            nc.scalar.mul(v8s, pv, ivw)
                        else:
                            nc.vector.tensor_scalar_mul(out=v8s, in0=pv,
                                                        scalar1=ivw)
                        nc.gpsimd.dma_start(
                            out=v8p[64 * (mt % 2):64 * (mt % 2) + 64,
                                    mt // 8, (mt % 8) // 2, :, :],
                            in_=v8s)

                for mc in range(4):
                    for mt in range(mc * 4, mc * 4 + 4):
                        ph1_body(mt)
                    ph2_chunk(mc)

            # ---------------- phase 3: attention per (b, h) ------------------
            with tc.tile_pool(name="sb3", bufs=4) as sb3, \
                 tc.tile_pool(name="e8p", bufs=3) as e8p, \
                 tc.tile_pool(name="atp", bufs=3) as atp, \
                 tc.tile_pool(name="recp", bufs=2) as recp, \
                 tc.tile_pool(name="psx", bufs=2, space="PSUM") as psx, \
                 tc.tile_pool(name="psc", bufs=3, space="PSUM") as psc, \
                 tc.tile_pool(name="psav", bufs=1, space="PSUM") as psav:

                at_tiles = {}
                rec_tiles = {}
                rbf_tiles = {}

                def stage_a(bh, mt):
                    b, h = divmod(bh, H)
                    hh, po = h // 2, (h % 2) * 64
                    hsl = slice(po, po + 64)
                    mg = slice(b * T + mt * P, b * T + (mt + 1) * P)
                    if mt % 2 == 0:
                        xbf_box[0] = xbf_ring[(bh * 4 + mt // 2) % 3]
                    xbf = xbf_box[0][:, mt % 2, :]
                    for nck in range(2):
                        xp = psx.tile([P, 512], F32, tag="x", bufs=3)
                        nc.tensor.matmul(
                            xp, lhsT=qv8[hh][hsl, mg],
                            rhs=p8[hh][hsl, b * T + nck * 512:b * T + (nck + 1) * 512],
                            start=True, stop=True)
                        nc.vector.tensor_copy(
                            out=xbf[:, 1 + nck * 512:1 + (nck + 1) * 512], in_=xp)
                    if mt % 2 == 1:
                        nc.sync.dma_start(
                            out=bass.AP(
                                tensor=scr[bh % NSCR].tensor,
                                offset=(mt - 1) * P * (T + 1),
                                ap=[[T + 1, P], [P * (T + 1), 2], [1, T + 1]]),
                            in_=xbf_box[0])
                        issue_shear(bh, mt - 1)
                        if mt >= 3:
                            issue_shear(bh, mt - 2)
                        if mt == 7:
                            issue_shear(bh, 7)

                def issue_shear(bh, mt):
                    sc_t = scr[bh % NSCR]
                    t = sb3.tile([P, T], BF16, tag="rbf", name="rbf", bufs=8)
                    nc.gpsimd.dma_start(
                        out=t,
                        in_=bass.AP(tensor=sc_t.tensor,
                                    offset=T + mt * P * T,
                                    ap=[[T, P], [1, T]]))
                    rbf_tiles[(bh, mt)] = t

                def stage_b(bh, mt):
                    b, h = divmod(bh, H)
                    hh, po = h // 2, (h % 2) * 64
                    hsl = slice(po, po + 64)
                    mg = slice(b * T + mt * P, b * T + (mt + 1) * P)
                    at = at_tiles[bh]
                    rec2 = rec_tiles[bh]
                    rbf = rbf_tiles.pop((bh, mt))
                    if mt % 4 == 0:
                        e8q_box[0] = e8p.tile([P, 4, T], FP8, tag="E8",
                                              name="E8")
                    e8 = e8q_box[0][:, mt % 4, :]
                    # chunk 0: content matmul, +R on DVE, exp from SBUF
                    cp0 = psc.tile([P, 512], F32, tag="c", name="cp0")
                    nc.tensor.matmul(
                        cp0, lhsT=qu8[hh][hsl, mg],
                        rhs=k8[hh][hsl, b * T:b * T + 512],
                        start=True, stop=True)
                    lbf = sb3.tile([P, 512], BF16, tag="lbf")
                    nc.vector.scalar_tensor_tensor(
                        out=lbf, in0=cp0, scalar=0.0, in1=rbf[:, 0:512],
                        op0=BYPASS, op1=ADD)
                    nc.scalar.activation(
                        out=e8[:, 0:512], in_=lbf,
                        func=EXP, scale=0.125,
                        accum_out=rec2[:, 0, mt:mt + 1])
                    # chunk 1: content matmul, +R via identity matmul, exp from PSUM
                    cp1 = psc.tile([P, 512], F32, tag="c", name="cp1")
                    nc.tensor.matmul(
                        cp1, lhsT=qu8[hh][hsl, mg],
                        rhs=k8[hh][hsl, b * T + 512:b * T + 1024],
                        start=True, stop=False, skip_group_check=True)
                    nc.tensor.matmul(
                        cp1, lhsT=id_bf, rhs=rbf[:, 512:1024],
                        start=False, stop=True, skip_group_check=True)
                    nc.scalar.activation(
                        out=e8[:, 512:1024], in_=cp1,
                        func=EXP, scale=0.125,
                        accum_out=rec2[:, 1, mt:mt + 1])
                    if mt % 4 == 3:
                        nc.sync.dma_start_transpose(
                            out=at[:, mt // 4, :, :],
                            in_=e8q_box[0].bitcast(U16))

                def stage_d_pre(bh):
                    # softmax recip broadcast for bh (rec2 complete one step ago)
                    rec2 = rec_tiles.pop(bh)
                    recr = recp.tile([P, 8], F32, tag="recr")
                    nc.vector.tensor_tensor(out=recr, in0=rec2[:, 0, :],
                                            in1=rec2[:, 1, :], op=ADD)
                    nc.vector.reciprocal(out=recr, in_=recr)
                    rt = psx.tile([8, P], F32, tag="misc", bufs=1, name="rt")
                    nc.tensor.transpose(rt, recr, ident)
                    recT = recp.tile([8, P], BF16, tag="recT")
                    nc.scalar.copy(out=recT, in_=rt)
                    recF = recp.tile([1, T], BF16, tag="recF")
                    nc.gpsimd.dma_start(out=recF, in_=recT)
                    recB = recp.tile([P, T], BF16, tag="recB")
                    nc.gpsimd.partition_broadcast(recB, recF)
                    return recB

                def stage_d_piece(bh, i, recB, avp_box):
                    b, h = divmod(bh, H)
                    mc, c = i // 4, i % 4
                    if c == 0:
                        avp_box[0] = psav.tile([64, 512], F32, tag="av",
                                               name="av")
                    avp = avp_box[0]
                    raw = at_tiles[bh].bitcast(FP8)
                    rhs = bass.AP(
                        tensor=raw.tensor,
                        offset=raw.offset + mc * 4096 + c * 256,
                        ap=[[raw.ap[0][0], P], [1, 2], [1024, 4], [2, P]])
                    nc.tensor.matmul(
                        avp,
                        lhsT=v8p[:, b, c, :, 64 * h:64 * h + 64],
                        rhs=rhs, start=(c == 0), stop=(c == 3),
                        perf_mode=DR, skip_group_check=True)
                    if c == 3:
                        nc.vector.scalar_tensor_tensor(
                            out=avT[h][0:64,
                                       b * T + mc * 512:b * T + (mc + 1) * 512],
                            in0=avp, scalar=0.0,
                            in1=recB[0:64, mc * 512:(mc + 1) * 512],
                            op0=BYPASS, op1=MULT)
                        if mc == 1:
                            del at_tiles[bh]

                def ph4_body(mt):
                    op = psx.tile([P, DM], F32, tag="misc", bufs=1, name="op")
                    for h_ in range(4):
                        nc.tensor.matmul(op,
                                         lhsT=avT[h_][:, mt * P:(mt + 1) * P],
                                         rhs=wo_sb[h_],
                                         start=(h_ == 0), stop=(h_ == 3))
                    ot = sb3.tile([P, DM], F32, tag="ot")
                    nc.vector.scalar_tensor_tensor(out=ot, in0=op, scalar=0.0,
                                                   in1=x_res[:, mt, :],
                                                   op0=BYPASS, op1=ADD)
                    nc.vector.tensor_tensor(out=ot, in0=ot, in1=bo_b, op=ADD)
                    nc.sync.dma_start(out=out[mt * P:(mt + 1) * P, :], in_=ot)

                avp_box = [None]
                e8q_box = [None]
                xbf_box = [None]
                recB_cur = None
                for step in range(NBH + 2):
                    if 0 <= step - 1 < NBH:
                        at_tiles[step - 1] = atp.tile([P, 2, 16, P], U16,
                                                      tag="at", name="at")
                        rec_tiles[step - 1] = recp.tile([P, 2, 8], F32,
                                                        tag="rec2",
                                                        name="rec2")
                    if step - 2 >= 0:
                        recB_cur = stage_d_pre(step - 2)
                    for mt in range(8):
                        if step < NBH:
                            stage_a(step, mt)
                        if 0 <= step - 1 < NBH:
                            stage_b(step - 1, mt)
                        if step - 2 >= 0:
                            stage_d_piece(step - 2, mt, recB_cur, avp_box)
                    if step - 2 == 3:
                        for mt in range(8):
                            ph4_body(mt)
                for mt in range(8, 16):
                    ph4_body(mt)
    nc.finalize()
    return nc


_NC = None


def make_in_maps(inputs):
    f = lambda a: np.ascontiguousarray(np.asarray(a, dtype=np.float32))
    x = f(inputs["inputs"]).reshape(B, T, DM)
    pos = f(inputs["pos_enc"]).reshape(B, T, DM)
    wq0 = f(inputs["Wq"]).reshape(DM, DM)
    wk0 = f(inputs["Wk"]).reshape(DM, DM)
    wv0 = f(inputs["Wv"]).reshape(DM, DM)
    wp0 = f(inputs["Wp"]).reshape(DM, DM)
    wo0 = f(inputs["Wo"]).reshape(DM, DM)
    gamma = f(inputs["gamma"]).reshape(DM, 1)
    beta = f(inputs["beta"]).reshape(DM)

    # fold LN gamma into x-side weights, beta into biases, bv through the
    # (normalized) attention into the output bias
    wq, wk, wv = gamma * wq0, gamma * wk0, gamma * wv0
    bqu = (f(inputs["bq"]).reshape(DM) + f(inputs["pos_bias_u"]).reshape(DM)
           + beta @ wq0)
    bqv = (f(inputs["bq"]).reshape(DM) + f(inputs["pos_bias_v"]).reshape(DM)
           + beta @ wq0)
    bk = f(inputs["bk"]).reshape(DM) + beta @ wk0
    bv_eff = f(inputs["bv"]).reshape(DM) + beta @ wv0
    bo = f(inputs["bo"]) + bv_eff @ wo0

    c = np.ascontiguousarray
    wq_dr = c((wq * WSCALE).reshape(2, P, DM))
    wk_dr = c((wk * WSCALE).reshape(2, P, DM))
    wp_dr = c((wp0 * WSCALE).reshape(2, P, DM))
    wv_dr = c((wv * WSCALE).reshape(2, P, DM))
    wo_dr = c(wo0.reshape(4, 64, DM))
    dqv = bqv - bqu
    bvec = c(np.stack([bqu[:P], bqu[P:], dqv[:P], dqv[P:],
                       bk[:P], bk[P:]]))

    shared = dict(wq=wq_dr, wk=wk_dr, wp=wp_dr, wv=wv_dr, wo=wo_dr,
                  bvec=bvec, bo=c(bo))
    in_maps = []
    for core in range(NCORES):
        sl = slice(core * BL, (core + 1) * BL)
        in_maps.append(dict(
            x=c(x[sl].reshape(M, DM)),
            pos=c(pos[sl].reshape(M, DM)),
            **shared,
        ))
    return in_maps


def kernel(**inputs) -> np.ndarray:
    global _NC
    if _NC is None:
        _NC = build_bass()
    in_maps = make_in_maps(inputs)
    res = run_bass_kernel_spmd(_NC, in_maps, core_ids=list(range(NCORES)))
    outs = [r["out"].reshape(BL, T, DM) for r in res.results]
    return np.concatenate(outs, axis=0)
